# revision 22
# baseline (speedup 1.0000x reference)
"""Trainium2 Bass kernel for nn_BevEncode (DCNv2-style deformable conv), v5.

Per-core (8 cores = 2 batches x 4 group-quarters, 16 groups each):
  P1 conv (PE, fp16 in / fp32 psum): offset/mask conv3x3 stride2 -> offT2
     fp16 (bias fused into ACT psum-evacuation; mask sigmoid at select).
  P2 select: bilinear gather, separable, with (c,wo)-fused field layout:
     host preps Q/D row-fields as [group, row, plane(7), c(2), wo(128)] so
     each tap op covers both channels in one instruction:
       tap ops are [3ki, 3kj, 2c*128wo] = 2304-element fp16 TTs (2x DVE).
     y-axis difference-form: acc_v = Q_v + sum_j clamp(dy, j-3, j-2)*D_{j,v}
     x-axis: 5 hat weights on ACT, val = sum_v hx_v * acc_v (hx c-bcast).
     Final: mask (sigmoid, c-bcast), wg multiply [2oc,9k,2c*128], c-fold,
     k-tree reduction.

Self-contained: hardcodes shapes for B=2, C=128, H=W=256, G=64, K=9, stride 2.
"""

import sys
import os
import numpy as np

sys.path.insert(0, "/opt/trn_rl_repo")

B, C, H, W = 2, 128, 256, 256
G, KH, KW, KK = 64, 3, 3, 9
HO = WO = 128
GPC = 16            # groups per core
NCORES = 8
NPASS = 4           # conv output passes, 4 groups each
GPP = 4             # groups per pass
COPP = GPP * 27     # 108 conv out-channels per pass (4x18 off then 4x9 mask)
NOFF = GPP * 18     # 72 offset rows per pass
NCHUNK = 16         # conv spatial chunks
CHO = 8             # output rows per chunk
NBANK = CHO * WO // 512
SLABR, SLABC = 2 * CHO + 1, 260  # conv slab rows/cols (data at 2..257)
NPLANE = 7          # const planes (-3..3)
NQ = 3              # Q slots (rows 2p-3 .. 2p-1)
ND = 6              # D slots (rows 2p-2 .. 2p+3)
NSLOT = NQ + ND
XGROWS = 262        # padded row-fields: actual rows -3..258 at store 0..261
RPITCH = NPLANE * 2 * WO   # 1792 elements per stored row (plane, c, wo)
CW = 2 * WO         # 256: fused (c, wo) span

_PROGRAM_CACHE = {}


def build_program():
    import concourse.bass as bass
    import concourse.bacc as bacc
    import concourse.tile as tile
    from concourse import mybir

    f32 = mybir.dt.float32
    f16 = mybir.dt.float16
    AF = mybir.ActivationFunctionType
    OP = mybir.AluOpType

    nc = bacc.Bacc("TRN2", target_bir_lowering=False, debug=False)

    x_in = nc.dram_tensor("xconv", [C, H, W], f16, kind="ExternalInput")
    qf = nc.dram_tensor("qf", [GPC, XGROWS, RPITCH], f16, kind="ExternalInput")
    df = nc.dram_tensor("df", [GPC, XGROWS, RPITCH], f16, kind="ExternalInput")
    wconv = nc.dram_tensor("wconv", [C, KK, NPASS * COPP], f16, kind="ExternalInput")
    bias_in = nc.dram_tensor("biasv", [NPASS * COPP], f32, kind="ExternalInput")
    wg_in = nc.dram_tensor("wgv", [GPC, 2 * KK * CW], f16, kind="ExternalInput")
    y_out = nc.dram_tensor("y", [2 * GPC, HO, WO], f32, kind="ExternalOutput")
    offT2 = nc.dram_tensor("offT2", [NPASS * COPP, HO, WO], f16, kind="Internal")

    def dram_ap(t, off, dims):
        a = t[:]
        return bass.AP(tensor=a.tensor, offset=a.offset + off,
                       ap=[list(d) for d in dims])

    def tile_ap(tt, off, dims):
        a = tt[:]
        return bass.AP(tensor=a.tensor, offset=a.offset + off,
                       ap=[list(a.ap[0])] + [list(d) for d in dims])

    with tile.TileContext(nc) as tc:
        import contextlib
        ctx = contextlib.ExitStack()
        with ctx:
            const_p = ctx.enter_context(tc.tile_pool(name="const", bufs=1))
            slab_p = ctx.enter_context(tc.tile_pool(name="slab", bufs=2))
            convo_p = ctx.enter_context(tc.tile_pool(name="convo", bufs=2))
            psum_p = ctx.enter_context(tc.tile_pool(name="psum", bufs=4, space="PSUM"))
            od_p = ctx.enter_context(tc.tile_pool(name="od", bufs=2))
            xrc_p = ctx.enter_context(tc.tile_pool(name="xrc", bufs=2))
            wgr_p = ctx.enter_context(tc.tile_pool(name="wgr", bufs=1))
            hat_p = ctx.enter_context(tc.tile_pool(name="hat", bufs=1))
            work_p = ctx.enter_context(tc.tile_pool(name="work", bufs=1))
            out_p = ctx.enter_context(tc.tile_pool(name="outb", bufs=1))

            # ---- constants ----
            bias_sb = const_p.tile([128, NPASS], f32)
            nc.sync.dma_start(
                out=bias_sb[:COPP, :],
                in_=dram_ap(bias_in, 0, [[1, COPP], [COPP, NPASS]]))
            hatc = const_p.tile([128, 6], f32)
            for i, bv in enumerate([2.0, 1.0, 0.0, -1.0, -2.0, 1.0]):
                nc.vector.memset(hatc[:, i:i + 1], bv)

            def conv_pass(p):
                co0 = p * COPP
                wsb = slab_p.tile([C, KK, COPP], f16, tag="wsb",
                                  name=f"wsb{p}")
                nc.sync.dma_start(
                    out=wsb[:],
                    in_=dram_ap(wconv, co0,
                                [[KK * NPASS * COPP, C], [NPASS * COPP, KK],
                                 [1, COPP]]))
                for chn in range(NCHUNK):
                    ho0 = chn * CHO
                    slab = slab_p.tile([C, SLABR, SLABC], f16, tag="slab",
                                       name=f"slab_{p}_{chn}")
                    r0 = 2 * ho0 - 1
                    rlo = max(r0, 0)
                    rn = min(r0 + SLABR, H) - rlo
                    if r0 < 0:
                        nc.vector.memset(slab[:, 0, :], 0)
                    nc.vector.memset(slab[:, :, 0:2], 0)
                    nc.vector.memset(slab[:, :, 258:260], 0)
                    nc.sync.dma_start(
                        out=slab[:, rlo - r0:rlo - r0 + rn, 2:258],
                        in_=dram_ap(x_in, rlo * W, [[H * W, C], [W, rn], [1, W]]))
                    convo = convo_p.tile([COPP, CHO * WO], f16, tag="convo",
                                         name=f"convo_{p}_{chn}")
                    pss = [psum_p.tile([128, 512], f32, tag="ps",
                                       name=f"ps_{p}_{chn}_{bank}")
                           for bank in range(NBANK)]
                    for kk in range(KK):
                        ki, kj = kk // 3, kk % 3
                        for bank in range(NBANK):
                            rhs = tile_ap(slab, (8 * bank + ki) * SLABC + kj + 1,
                                          [[2 * SLABC, 4], [2, WO]])
                            nc.tensor.matmul(out=pss[bank][:COPP, :],
                                             lhsT=wsb[:, kk, :],
                                             rhs=rhs,
                                             start=(kk == 0), stop=(kk == KK - 1))
                    for bank in range(NBANK):
                        nc.scalar.activation(
                            out=convo[:COPP, bank * 512:(bank + 1) * 512],
                            in_=pss[bank][:COPP, :], func=AF.Identity,
                            bias=bias_sb[:COPP, p:p + 1], scale=1.0)
                    nc.sync.dma_start(
                        out=dram_ap(offT2, co0 * HO * WO + ho0 * WO,
                                    [[HO * WO, COPP], [1, CHO * WO]]),
                        in_=convo[:])

            def select_group(g, outbuf):
                p = g // GPP
                gl = g % GPP
                co_off = p * COPP + gl * 27

                odo = od_p.tile([128, 27, WO], f16, tag="odo", name=f"odo{g}")
                odm = work_p.tile([128, KK, 2, WO], f16, tag="odm", name=f"odm{g}")
                nc.sync.dma_start(out=odo[:], in_=dram_ap(
                    offT2, co_off * HO * WO,
                    [[WO, 128], [HO * WO, 27], [1, WO]]))
                dy_ap = tile_ap(odo, 0, [[2 * WO, KK], [1, WO]])
                dx_ap = tile_ap(odo, WO, [[2 * WO, KK], [1, WO]])
                odmr_ap = tile_ap(odo, 18 * WO, [[WO, KK], [1, WO]])
                # odm c-duplicated [KK, 2, WO] via two per-c sigmoids
                for c in range(2):
                    nc.scalar.activation(
                        out=tile_ap(odm, c * WO, [[CW, KK], [1, WO]]),
                        in_=odmr_ap, func=AF.Sigmoid,
                        bias=hatc[:, 2:3], scale=1.0)

                # xrc2: [slot(9), plane(7), c(2), wo(128)] per partition
                xrc = xrc_p.tile([128, NSLOT, RPITCH], f16, tag="xrc",
                                 name=f"xrc{g}")
                nc.gpsimd.dma_start(
                    out=tile_ap(xrc, 0, [[RPITCH, NQ], [1, RPITCH]]),
                    in_=dram_ap(qf, g * XGROWS * RPITCH,
                                [[2 * RPITCH, 128], [RPITCH, NQ], [1, RPITCH]]))
                nc.gpsimd.dma_start(
                    out=tile_ap(xrc, NQ * RPITCH, [[RPITCH, ND], [1, RPITCH]]),
                    in_=dram_ap(df, g * XGROWS * RPITCH + RPITCH,
                                [[2 * RPITCH, 128], [RPITCH, ND], [1, RPITCH]]))

                wgr = wgr_p.tile([128, 2, KK, CW], f16, tag="wgr",
                                 name=f"wgr{g}")
                nc.scalar.dma_start(
                    out=wgr[:],
                    in_=dram_ap(wg_in, g * 2 * KK * CW,
                                [[0, 128], [1, 2 * KK * CW]]))

                # ty2_j = clamp(dy, j-3, j-2), c-duplicated: [KK, 2, WO].
                # Two plain per-c TS writes: a stride-0 broadcast input on
                # tensor_scalar races with the 2-port DVE perf modes on HW.
                ty2 = []
                for j in range(1, 5):
                    t = work_p.tile([128, KK, 2, WO], f16, tag=f"ty{j}",
                                    name=f"ty{g}_{j}")
                    for c in range(2):
                        nc.vector.tensor_scalar(
                            out=tile_ap(t, c * WO, [[CW, KK], [1, WO]]),
                            in0=dy_ap,
                            scalar1=float(j - 3), scalar2=float(j - 2),
                            op0=OP.max, op1=OP.min)
                    ty2.append(t)

                def make_hx(v, h, t1):
                    # hat(dx - v), c-duplicated [KK, 2, WO]
                    for c in range(2):
                        nc.scalar.activation(
                            out=tile_ap(t1, c * WO, [[CW, KK], [1, WO]]),
                            in_=dx_ap, func=AF.Abs,
                            bias=hatc[:, v + 2:v + 3], scale=1.0)
                    nc.scalar.activation(out=h[:], in_=t1[:], func=AF.Relu,
                                         bias=hatc[:, 5:6], scale=-1.0)

                # v-group tap tiles: [3v, KK, 2, WO]
                acc2 = work_p.tile([128, 3, KK, 2, WO], f16, tag="acc2",
                                   name=f"acc2_{g}")
                tmp2 = work_p.tile([128, 3, KK, 2, WO], f16, tag="tmp2",
                                   name=f"tmp2_{g}")
                val = work_p.tile([128, KK, 2, WO], f16, tag="val",
                                  name=f"val{g}")
                PCW = KK * CW      # 2304: one v-plane span

                def dtap2(j, v, nv):
                    return tile_ap(xrc, (j + 2) * RPITCH + (v + 2) * CW,
                                   [[CW, nv], [RPITCH, 3], [1, 3 * CW]])

                def qtap2(v, nv):
                    return tile_ap(xrc, (v + 2) * CW,
                                   [[CW, nv], [RPITCH, 3], [1, 3 * CW]])

                def ty_ap2(t, nv):
                    return tile_ap(t, 0, [[0, nv], [3 * CW, 3], [1, 3 * CW]])

                val_v3 = tile_ap(val, 0, [[CW, KK], [WO, 2], [1, WO]])

                first_v = True
                for va, nv in ((-2, 3), (1, 2)):
                    a_k = tile_ap(acc2, 0, [[PCW, nv], [3 * CW, 3], [1, 3 * CW]])
                    t_k = tile_ap(tmp2, 0, [[PCW, nv], [3 * CW, 3], [1, 3 * CW]])
                    a_f = tile_ap(acc2, 0, [[1, nv * PCW]])
                    t_f = tile_ap(tmp2, 0, [[1, nv * PCW]])
                    nc.vector.tensor_tensor(out=a_k, in0=ty_ap2(ty2[0], nv),
                                            in1=dtap2(1, va, nv), op=OP.mult)
                    for j in (2, 3, 4):
                        nc.vector.tensor_tensor(out=t_k,
                                                in0=ty_ap2(ty2[j - 1], nv),
                                                in1=dtap2(j, va, nv),
                                                op=OP.mult)
                        nc.vector.tensor_tensor(out=a_f, in0=a_f,
                                                in1=t_f, op=OP.add)
                    nc.vector.tensor_tensor(out=a_k, in0=a_k,
                                            in1=qtap2(va, nv), op=OP.add)
                    for vi in range(nv):
                        v = va + vi
                        h = hat_p.tile([128, KK, 2, WO], f16, tag=f"hx{vi}",
                                       name=f"hx{g}_{v}")
                        t1 = hat_p.tile([128, KK, 2, WO], f16, tag="hxt",
                                        name=f"hxt{g}_{v}")
                        make_hx(v, h, t1)
                        a_slice = tile_ap(acc2, vi * PCW,
                                          [[CW, KK], [WO, 2], [1, WO]])
                        if first_v:
                            nc.vector.tensor_tensor(out=val_v3, in0=h[:],
                                                    in1=a_slice, op=OP.mult)
                            first_v = False
                        else:
                            t_slice = tile_ap(tmp2, vi * PCW,
                                              [[CW, KK], [WO, 2], [1, WO]])
                            nc.vector.tensor_tensor(out=t_slice, in0=h[:],
                                                    in1=a_slice, op=OP.mult)
                            nc.vector.tensor_tensor(
                                out=tile_ap(val, 0, [[1, PCW]]),
                                in0=tile_ap(val, 0, [[1, PCW]]),
                                in1=tile_ap(tmp2, vi * PCW, [[1, PCW]]),
                                op=OP.add)

                # mask multiply (c-duplicated odm)
                nc.vector.tensor_tensor(out=val_v3, in0=val_v3,
                                        in1=tile_ap(odm, 0, [[CW, KK],
                                                             [WO, 2], [1, WO]]),
                                        op=OP.mult)

                # wg multiply: tts[oc, k, (c,wo)] = val[k,(c,wo)] * wgr
                tts = work_p.tile([128, 2, KK, CW], f16, tag="tts",
                                  name=f"tts{g}")
                nc.vector.tensor_tensor(
                    out=tts[:],
                    in0=tile_ap(val, 0, [[0, 2], [CW, KK], [1, CW]]),
                    in1=wgr[:], op=OP.mult)
                # c-fold + k-tree, in place within tts (out == in0 region)
                def tts_v(koff, nk, coff=0):
                    return tile_ap(tts, koff * CW + coff,
                                   [[KK * CW, 2], [CW, nk], [1, WO]])
                nc.vector.tensor_tensor(out=tts_v(0, KK), in0=tts_v(0, KK),
                                        in1=tts_v(0, KK, WO), op=OP.add)
                nc.vector.tensor_tensor(out=tts_v(0, 4), in0=tts_v(0, 4),
                                        in1=tts_v(4, 4), op=OP.add)
                nc.vector.tensor_tensor(out=tts_v(0, 2), in0=tts_v(0, 2),
                                        in1=tts_v(2, 2), op=OP.add)
                nc.vector.tensor_tensor(out=tts_v(0, 1), in0=tts_v(0, 1),
                                        in1=tts_v(1, 1), op=OP.add)
                nc.vector.tensor_tensor(out=outbuf[:, 2 * gl:2 * gl + 2, :],
                                        in0=tts_v(0, 1), in1=tts_v(8, 1),
                                        op=OP.add)

            for p in range(NPASS):
                conv_pass(p)
                outbuf = out_p.tile([128, 2 * GPP, WO], f32, tag="outbuf",
                                    name=f"outbuf{p}")
                for gl in range(GPP):
                    select_group(p * GPP + gl, outbuf)
                nc.scalar.dma_start(
                    out=dram_ap(y_out, p * GPP * 2 * HO * WO,
                                [[WO, 128], [HO * WO, 2 * GPP], [1, WO]]),
                    in_=outbuf[:])

    nc.compile()
    return nc


def _host_prep(inputs):
    x = np.asarray(inputs["x"], dtype=np.float32)
    w_offset = np.asarray(inputs["w_offset"], dtype=np.float32)
    b_offset = np.asarray(inputs["b_offset"], dtype=np.float32)
    w_mask = np.asarray(inputs["w_mask"], dtype=np.float32)
    b_mask = np.asarray(inputs["b_mask"], dtype=np.float32)
    w_deform = np.asarray(inputs["w_deform"], dtype=np.float32)

    in_maps = []
    for core in range(NCORES):
        b = core // 4
        q = core % 4
        gs = np.arange(GPC) + q * GPC
        wrows, brows = [], []
        for p in range(NPASS):
            for gl in range(GPP):
                g = gs[p * GPP + gl]
                idx = np.arange(18) + g * KK * 2
                wrows.append(w_offset[idx])
                brows.append(b_offset[idx])
                idx = np.arange(KK) + g * KK
                wrows.append(w_mask[idx])
                brows.append(b_mask[idx])
        wall = np.concatenate(wrows, axis=0)
        ball = np.ascontiguousarray(np.concatenate(brows, axis=0))
        # rotate input channels so this core's 32 group-channels are first
        perm = np.r_[np.arange(32 * q, C), np.arange(0, 32 * q)]
        wconv = np.ascontiguousarray(
            wall.reshape(432, C, KK)[:, perm, :].transpose(1, 2, 0)
        ).astype(np.float16)
        xb = x[b][perm]
        xconv = np.ascontiguousarray(xb).astype(np.float16)

        # Q/D row-fields for this core's 32 channels, plane layout
        # [ch, row, plane, wo]. Work in fp32 on fp16-rounded x.
        x32 = xb[:32].astype(np.float16).astype(np.float32)
        rpad = 7
        xp = np.zeros((32, H + 2 * rpad, W), np.float32)
        xp[:, rpad:rpad + H, :] = x32
        rows = np.arange(-3, 259) + rpad
        xr0 = xp[:, rows, :]
        xr1 = xp[:, rows + 1, :]
        xr2 = xp[:, rows + 2, :]
        xr3 = xp[:, rows + 3, :]
        xr4 = xp[:, rows + 4, :]
        xrm1 = xp[:, rows - 1, :]
        qrow = (-xr0 + xr1 + xr2 + xr3 - xr4)          # [32, 262, W]
        drow = (xr0 - xrm1)
        cpad = 4

        def to_planes(f):
            fp = np.zeros((32, XGROWS, W + 2 * cpad), np.float32)
            fp[:, :, cpad:cpad + W] = f
            out = np.empty((32, XGROWS, NPLANE, WO), np.float32)
            for ci, cs in enumerate(range(-3, 4)):
                out[:, :, ci, :] = fp[:, :, cpad + cs:cpad + cs + 2 * WO:2]
            return out

        qpl = to_planes(qrow)   # [32, 262, 7, 128]
        dpl = to_planes(drow)
        # interleave channel pairs: [GPC, row, plane, c, wo]
        qi = np.empty((GPC, XGROWS, NPLANE, 2, WO), np.float32)
        di = np.empty((GPC, XGROWS, NPLANE, 2, WO), np.float32)
        for gl in range(GPC):
            qi[gl, :, :, 0] = qpl[2 * gl]
            qi[gl, :, :, 1] = qpl[2 * gl + 1]
            di[gl, :, :, 0] = dpl[2 * gl]
            di[gl, :, :, 1] = dpl[2 * gl + 1]
        qfv = np.ascontiguousarray(
            qi.reshape(GPC, XGROWS, RPITCH)).astype(np.float16)
        dfv = np.ascontiguousarray(
            di.reshape(GPC, XGROWS, RPITCH)).astype(np.float16)

        # wg: [GPC, oc, k, c, wo]
        wg = w_deform.reshape(G, 2, 2, KK)[gs]      # [GPC, oc, c, k]
        wgv = np.ascontiguousarray(
            np.broadcast_to(
                wg.transpose(0, 1, 3, 2)[:, :, :, :, None],
                (GPC, 2, KK, 2, WO)).reshape(GPC, 2 * KK * CW)
        ).astype(np.float16)
        in_maps.append({
            "xconv": xconv,
            "qf": qfv,
            "df": dfv,
            "wconv": wconv,
            "biasv": ball,
            "wgv": wgv,
        })
    return in_maps


def kernel(**inputs):
    from concourse.bass_utils import run_bass_kernel_spmd

    if "prog" not in _PROGRAM_CACHE:
        _PROGRAM_CACHE["prog"] = build_program()
    nc = _PROGRAM_CACHE["prog"]
    in_maps = _host_prep(inputs)
    res = run_bass_kernel_spmd(nc, in_maps, list(range(NCORES)),
                               trace=bool(int(os.environ.get("BEV_TRACE", "0"))))
    _PROGRAM_CACHE["last_result"] = res
    out = np.empty((B, C, HO, WO), dtype=np.float32)
    for core in range(NCORES):
        b = core // 4
        q = core % 4
        out[b, q * 32:(q + 1) * 32] = res.results[core]["y"]
    return out


# revision 23
# speedup vs baseline: 1.1757x; 1.1757x over previous
"""Trainium2 Bass kernel for nn_BevEncode (DCNv2-style deformable conv), v5.

Per-core (8 cores = 2 batches x 4 group-quarters, 16 groups each):
  P1 conv (PE, fp16 in / fp32 psum): offset/mask conv3x3 stride2 -> offT2
     fp16 (bias fused into ACT psum-evacuation; mask sigmoid at select).
  P2 select: bilinear gather, separable, with (c,wo)-fused field layout:
     host preps Q/D row-fields as [group, row, plane(7), c(2), wo(128)] so
     each tap op covers both channels in one instruction:
       tap ops are [3ki, 3kj, 2c*128wo] = 2304-element fp16 TTs (2x DVE).
     y-axis difference-form: acc_v = Q_v + sum_j clamp(dy, j-3, j-2)*D_{j,v}
     x-axis: 5 hat weights on ACT, val = sum_v hx_v * acc_v (hx c-bcast).
     Final: mask (sigmoid, c-bcast), wg multiply [2oc,9k,2c*128], c-fold,
     k-tree reduction.

Self-contained: hardcodes shapes for B=2, C=128, H=W=256, G=64, K=9, stride 2.
"""

import sys
import os
import numpy as np

sys.path.insert(0, "/opt/trn_rl_repo")

B, C, H, W = 2, 128, 256, 256
G, KH, KW, KK = 64, 3, 3, 9
HO = WO = 128
GPC = 16            # groups per core
NCORES = 8
NPASS = 4           # conv output passes, 4 groups each
GPP = 4             # groups per pass
COPP = GPP * 27     # 108 conv out-channels per pass (4x18 off then 4x9 mask)
NOFF = GPP * 18     # 72 offset rows per pass
NCHUNK = 16         # conv spatial chunks
CHO = 8             # output rows per chunk
NBANK = CHO * WO // 512
SLABR, SLABC = 2 * CHO + 1, 260  # conv slab rows/cols (data at 2..257)
NPLANE = 7          # const planes (-3..3)
NQ = 3              # Q slots (rows 2p-3 .. 2p-1)
ND = 6              # D slots (rows 2p-2 .. 2p+3)
NSLOT = NQ + ND
XGROWS = 262        # padded row-fields: actual rows -3..258 at store 0..261
RPITCH = NPLANE * 2 * WO   # 1792 elements per stored row (plane, c, wo)
CW = 2 * WO         # 256: fused (c, wo) span

_PROGRAM_CACHE = {}


def build_program():
    import concourse.bass as bass
    import concourse.bacc as bacc
    import concourse.tile as tile
    from concourse import mybir

    f32 = mybir.dt.float32
    f16 = mybir.dt.float16
    AF = mybir.ActivationFunctionType
    OP = mybir.AluOpType

    nc = bacc.Bacc("TRN2", target_bir_lowering=False, debug=False)

    x_in = nc.dram_tensor("xconv", [C, H, W], f16, kind="ExternalInput")
    qf = nc.dram_tensor("qf", [GPC, XGROWS, RPITCH], f16, kind="ExternalInput")
    df = nc.dram_tensor("df", [GPC, XGROWS, RPITCH], f16, kind="ExternalInput")
    wconv = nc.dram_tensor("wconv", [C, KK, NPASS * COPP], f16, kind="ExternalInput")
    bias_in = nc.dram_tensor("biasv", [NPASS * COPP], f32, kind="ExternalInput")
    wg_in = nc.dram_tensor("wgv", [GPC, 2 * KK * CW], f16, kind="ExternalInput")
    y_out = nc.dram_tensor("y", [2 * GPC, HO, WO], f32, kind="ExternalOutput")
    offT2 = nc.dram_tensor("offT2", [NPASS * COPP, HO, WO], f16, kind="Internal")

    def dram_ap(t, off, dims):
        a = t[:]
        return bass.AP(tensor=a.tensor, offset=a.offset + off,
                       ap=[list(d) for d in dims])

    def tile_ap(tt, off, dims):
        a = tt[:]
        return bass.AP(tensor=a.tensor, offset=a.offset + off,
                       ap=[list(a.ap[0])] + [list(d) for d in dims])

    with tile.TileContext(nc) as tc:
        import contextlib
        ctx = contextlib.ExitStack()
        with ctx:
            const_p = ctx.enter_context(tc.tile_pool(name="const", bufs=1))
            slab_p = ctx.enter_context(tc.tile_pool(name="slab", bufs=2))
            convo_p = ctx.enter_context(tc.tile_pool(name="convo", bufs=2))
            psum_p = ctx.enter_context(tc.tile_pool(name="psum", bufs=4, space="PSUM"))
            od_p = ctx.enter_context(tc.tile_pool(name="od", bufs=2))
            xrc_p = ctx.enter_context(tc.tile_pool(name="xrc", bufs=2))
            wgr_p = ctx.enter_context(tc.tile_pool(name="wgr", bufs=1))
            hat_p = ctx.enter_context(tc.tile_pool(name="hat", bufs=1))
            work_p = ctx.enter_context(tc.tile_pool(name="work", bufs=1))
            out_p = ctx.enter_context(tc.tile_pool(name="outb", bufs=1))

            # ---- constants ----
            bias_sb = const_p.tile([128, NPASS], f32)
            nc.sync.dma_start(
                out=bias_sb[:COPP, :],
                in_=dram_ap(bias_in, 0, [[1, COPP], [COPP, NPASS]]))
            hatc = const_p.tile([128, 6], f32)
            for i, bv in enumerate([2.0, 1.0, 0.0, -1.0, -2.0, 1.0]):
                nc.vector.memset(hatc[:, i:i + 1], bv)

            def conv_pass(p):
                co0 = p * COPP
                wsb = slab_p.tile([C, KK, COPP], f16, tag="wsb",
                                  name=f"wsb{p}")
                nc.sync.dma_start(
                    out=wsb[:],
                    in_=dram_ap(wconv, co0,
                                [[KK * NPASS * COPP, C], [NPASS * COPP, KK],
                                 [1, COPP]]))
                for chn in range(NCHUNK):
                    ho0 = chn * CHO
                    slab = slab_p.tile([C, SLABR, SLABC], f16, tag="slab",
                                       name=f"slab_{p}_{chn}")
                    r0 = 2 * ho0 - 1
                    rlo = max(r0, 0)
                    rn = min(r0 + SLABR, H) - rlo
                    if r0 < 0:
                        nc.vector.memset(slab[:, 0, :], 0)
                    nc.vector.memset(slab[:, :, 0:2], 0)
                    nc.vector.memset(slab[:, :, 258:260], 0)
                    nc.sync.dma_start(
                        out=slab[:, rlo - r0:rlo - r0 + rn, 2:258],
                        in_=dram_ap(x_in, rlo * W, [[H * W, C], [W, rn], [1, W]]))
                    convo = convo_p.tile([COPP, CHO * WO], f16, tag="convo",
                                         name=f"convo_{p}_{chn}")
                    pss = [psum_p.tile([128, 512], f32, tag="ps",
                                       name=f"ps_{p}_{chn}_{bank}")
                           for bank in range(NBANK)]
                    for kk in range(KK):
                        ki, kj = kk // 3, kk % 3
                        for bank in range(NBANK):
                            rhs = tile_ap(slab, (8 * bank + ki) * SLABC + kj + 1,
                                          [[2 * SLABC, 4], [2, WO]])
                            nc.tensor.matmul(out=pss[bank][:COPP, :],
                                             lhsT=wsb[:, kk, :],
                                             rhs=rhs,
                                             start=(kk == 0), stop=(kk == KK - 1))
                    for bank in range(NBANK):
                        nc.scalar.activation(
                            out=convo[:COPP, bank * 512:(bank + 1) * 512],
                            in_=pss[bank][:COPP, :], func=AF.Identity,
                            bias=bias_sb[:COPP, p:p + 1], scale=1.0)
                    nc.sync.dma_start(
                        out=dram_ap(offT2, co0 * HO * WO + ho0 * WO,
                                    [[HO * WO, COPP], [1, CHO * WO]]),
                        in_=convo[:])

            def select_group(g, outbuf):
                p = g // GPP
                gl = g % GPP
                co_off = p * COPP + gl * 18
                co_msk = p * COPP + NOFF + gl * 9

                odo = od_p.tile([128, 18, WO], f16, tag="odo", name=f"odo{g}")
                odmr = work_p.tile([128, KK, WO], f16, tag="odmr", name=f"odmr{g}")
                odm = work_p.tile([128, KK, 2, WO], f16, tag="odm", name=f"odm{g}")
                nc.sync.dma_start(out=odo[:], in_=dram_ap(
                    offT2, co_off * HO * WO,
                    [[WO, 128], [HO * WO, 18], [1, WO]]))
                nc.sync.dma_start(out=odmr[:], in_=dram_ap(
                    offT2, co_msk * HO * WO,
                    [[WO, 128], [HO * WO, KK], [1, WO]]))
                dy_ap = tile_ap(odo, 0, [[2 * WO, KK], [1, WO]])
                dx_ap = tile_ap(odo, WO, [[2 * WO, KK], [1, WO]])
                # odm c-duplicated [KK, 2, WO] via two per-c sigmoids
                for c in range(2):
                    nc.scalar.activation(
                        out=tile_ap(odm, c * WO, [[CW, KK], [1, WO]]),
                        in_=odmr[:], func=AF.Sigmoid,
                        bias=hatc[:, 2:3], scale=1.0)

                # xrc2: [slot(9), plane(7), c(2), wo(128)] per partition
                xrc = xrc_p.tile([128, NSLOT, RPITCH], f16, tag="xrc",
                                 name=f"xrc{g}")
                nc.gpsimd.dma_start(
                    out=tile_ap(xrc, 0, [[RPITCH, NQ], [1, RPITCH]]),
                    in_=dram_ap(qf, g * XGROWS * RPITCH,
                                [[2 * RPITCH, 128], [RPITCH, NQ], [1, RPITCH]]))
                nc.gpsimd.dma_start(
                    out=tile_ap(xrc, NQ * RPITCH, [[RPITCH, ND], [1, RPITCH]]),
                    in_=dram_ap(df, g * XGROWS * RPITCH + RPITCH,
                                [[2 * RPITCH, 128], [RPITCH, ND], [1, RPITCH]]))

                wgr = wgr_p.tile([128, 2, KK, CW], f16, tag="wgr",
                                 name=f"wgr{g}")
                nc.scalar.dma_start(
                    out=wgr[:],
                    in_=dram_ap(wg_in, g * 2 * KK * CW,
                                [[0, 128], [1, 2 * KK * CW]]))

                # ty2_j = clamp(dy, j-3, j-2), c-duplicated: [KK, 2, WO].
                # Two plain per-c TS writes: a stride-0 broadcast input on
                # tensor_scalar races with the 2-port DVE perf modes on HW.
                ty2 = []
                for j in range(1, 5):
                    t = work_p.tile([128, KK, 2, WO], f16, tag=f"ty{j}",
                                    name=f"ty{g}_{j}")
                    for c in range(2):
                        nc.vector.tensor_scalar(
                            out=tile_ap(t, c * WO, [[CW, KK], [1, WO]]),
                            in0=dy_ap,
                            scalar1=float(j - 3), scalar2=float(j - 2),
                            op0=OP.max, op1=OP.min)
                    ty2.append(t)

                def make_hx(v, h, t1):
                    # hat(dx - v), c-duplicated [KK, 2, WO]
                    for c in range(2):
                        nc.scalar.activation(
                            out=tile_ap(t1, c * WO, [[CW, KK], [1, WO]]),
                            in_=dx_ap, func=AF.Abs,
                            bias=hatc[:, v + 2:v + 3], scale=1.0)
                    nc.scalar.activation(out=h[:], in_=t1[:], func=AF.Relu,
                                         bias=hatc[:, 5:6], scale=-1.0)

                # v-group tap tiles: [3v, KK, 2, WO]
                acc2 = work_p.tile([128, 3, KK, 2, WO], f16, tag="acc2",
                                   name=f"acc2_{g}")
                tmp2 = work_p.tile([128, 3, KK, 2, WO], f16, tag="tmp2",
                                   name=f"tmp2_{g}")
                val = work_p.tile([128, KK, 2, WO], f16, tag="val",
                                  name=f"val{g}")
                PCW = KK * CW      # 2304: one v-plane span

                def dtap2(j, v, nv):
                    return tile_ap(xrc, (j + 2) * RPITCH + (v + 2) * CW,
                                   [[CW, nv], [RPITCH, 3], [1, 3 * CW]])

                def qtap2(v, nv):
                    return tile_ap(xrc, (v + 2) * CW,
                                   [[CW, nv], [RPITCH, 3], [1, 3 * CW]])

                def ty_ap2(t, nv):
                    return tile_ap(t, 0, [[0, nv], [3 * CW, 3], [1, 3 * CW]])

                val_v3 = tile_ap(val, 0, [[CW, KK], [WO, 2], [1, WO]])

                first_v = True
                for va, nv in ((-2, 3), (1, 2)):
                    a_k = tile_ap(acc2, 0, [[PCW, nv], [3 * CW, 3], [1, 3 * CW]])
                    t_k = tile_ap(tmp2, 0, [[PCW, nv], [3 * CW, 3], [1, 3 * CW]])
                    a_f = tile_ap(acc2, 0, [[1, nv * PCW]])
                    t_f = tile_ap(tmp2, 0, [[1, nv * PCW]])
                    nc.vector.tensor_tensor(out=a_k, in0=ty_ap2(ty2[0], nv),
                                            in1=dtap2(1, va, nv), op=OP.mult)
                    for j in (2, 3, 4):
                        nc.vector.tensor_tensor(out=t_k,
                                                in0=ty_ap2(ty2[j - 1], nv),
                                                in1=dtap2(j, va, nv),
                                                op=OP.mult)
                        nc.vector.tensor_tensor(out=a_f, in0=a_f,
                                                in1=t_f, op=OP.add)
                    nc.vector.tensor_tensor(out=a_k, in0=a_k,
                                            in1=qtap2(va, nv), op=OP.add)
                    for vi in range(nv):
                        v = va + vi
                        h = hat_p.tile([128, KK, 2, WO], f16, tag=f"hx{vi}",
                                       name=f"hx{g}_{v}")
                        t1 = hat_p.tile([128, KK, 2, WO], f16, tag="hxt",
                                        name=f"hxt{g}_{v}")
                        make_hx(v, h, t1)
                        a_slice = tile_ap(acc2, vi * PCW,
                                          [[CW, KK], [WO, 2], [1, WO]])
                        if first_v:
                            nc.vector.tensor_tensor(out=val_v3, in0=h[:],
                                                    in1=a_slice, op=OP.mult)
                            first_v = False
                        else:
                            t_slice = tile_ap(tmp2, vi * PCW,
                                              [[CW, KK], [WO, 2], [1, WO]])
                            nc.vector.tensor_tensor(out=t_slice, in0=h[:],
                                                    in1=a_slice, op=OP.mult)
                            nc.vector.tensor_tensor(
                                out=tile_ap(val, 0, [[1, PCW]]),
                                in0=tile_ap(val, 0, [[1, PCW]]),
                                in1=tile_ap(tmp2, vi * PCW, [[1, PCW]]),
                                op=OP.add)

                # mask multiply (c-duplicated odm)
                nc.vector.tensor_tensor(out=val_v3, in0=val_v3,
                                        in1=tile_ap(odm, 0, [[CW, KK],
                                                             [WO, 2], [1, WO]]),
                                        op=OP.mult)

                # wg multiply: tts[oc, k, (c,wo)] = val[k,(c,wo)] * wgr
                tts = work_p.tile([128, 2, KK, CW], f16, tag="tts",
                                  name=f"tts{g}")
                nc.vector.tensor_tensor(
                    out=tts[:],
                    in0=tile_ap(val, 0, [[0, 2], [CW, KK], [1, CW]]),
                    in1=wgr[:], op=OP.mult)
                # c-fold + k-tree, in place within tts (out == in0 region)
                def tts_v(koff, nk, coff=0):
                    return tile_ap(tts, koff * CW + coff,
                                   [[KK * CW, 2], [CW, nk], [1, WO]])
                nc.vector.tensor_tensor(out=tts_v(0, KK), in0=tts_v(0, KK),
                                        in1=tts_v(0, KK, WO), op=OP.add)
                nc.vector.tensor_tensor(out=tts_v(0, 4), in0=tts_v(0, 4),
                                        in1=tts_v(4, 4), op=OP.add)
                nc.vector.tensor_tensor(out=tts_v(0, 2), in0=tts_v(0, 2),
                                        in1=tts_v(2, 2), op=OP.add)
                nc.vector.tensor_tensor(out=tts_v(0, 1), in0=tts_v(0, 1),
                                        in1=tts_v(1, 1), op=OP.add)
                nc.vector.tensor_tensor(out=outbuf[:, 2 * gl:2 * gl + 2, :],
                                        in0=tts_v(0, 1), in1=tts_v(8, 1),
                                        op=OP.add)

            for p in range(NPASS):
                conv_pass(p)
                outbuf = out_p.tile([128, 2 * GPP, WO], f32, tag="outbuf",
                                    name=f"outbuf{p}")
                for gl in range(GPP):
                    select_group(p * GPP + gl, outbuf)
                nc.scalar.dma_start(
                    out=dram_ap(y_out, p * GPP * 2 * HO * WO,
                                [[WO, 128], [HO * WO, 2 * GPP], [1, WO]]),
                    in_=outbuf[:])

    nc.compile()
    return nc


def _host_prep(inputs):
    x = np.asarray(inputs["x"], dtype=np.float32)
    w_offset = np.asarray(inputs["w_offset"], dtype=np.float32)
    b_offset = np.asarray(inputs["b_offset"], dtype=np.float32)
    w_mask = np.asarray(inputs["w_mask"], dtype=np.float32)
    b_mask = np.asarray(inputs["b_mask"], dtype=np.float32)
    w_deform = np.asarray(inputs["w_deform"], dtype=np.float32)

    in_maps = []
    for core in range(NCORES):
        b = core // 4
        q = core % 4
        gs = np.arange(GPC) + q * GPC
        wrows, brows = [], []
        for p in range(NPASS):
            for gl in range(GPP):
                g = gs[p * GPP + gl]
                idx = np.arange(18) + g * KK * 2
                wrows.append(w_offset[idx])
                brows.append(b_offset[idx])
            for gl in range(GPP):
                g = gs[p * GPP + gl]
                idx = np.arange(KK) + g * KK
                wrows.append(w_mask[idx])
                brows.append(b_mask[idx])
        wall = np.concatenate(wrows, axis=0)
        ball = np.ascontiguousarray(np.concatenate(brows, axis=0))
        # rotate input channels so this core's 32 group-channels are first
        perm = np.r_[np.arange(32 * q, C), np.arange(0, 32 * q)]
        wconv = np.ascontiguousarray(
            wall.reshape(432, C, KK)[:, perm, :].transpose(1, 2, 0)
        ).astype(np.float16)
        xb = x[b][perm]
        xconv = np.ascontiguousarray(xb).astype(np.float16)

        # Q/D row-fields for this core's 32 channels, plane layout
        # [ch, row, plane, wo]. Work in fp32 on fp16-rounded x.
        x32 = xb[:32].astype(np.float16).astype(np.float32)
        rpad = 7
        xp = np.zeros((32, H + 2 * rpad, W), np.float32)
        xp[:, rpad:rpad + H, :] = x32
        rows = np.arange(-3, 259) + rpad
        xr0 = xp[:, rows, :]
        xr1 = xp[:, rows + 1, :]
        xr2 = xp[:, rows + 2, :]
        xr3 = xp[:, rows + 3, :]
        xr4 = xp[:, rows + 4, :]
        xrm1 = xp[:, rows - 1, :]
        qrow = (-xr0 + xr1 + xr2 + xr3 - xr4)          # [32, 262, W]
        drow = (xr0 - xrm1)
        cpad = 4

        def to_planes(f):
            fp = np.zeros((32, XGROWS, W + 2 * cpad), np.float32)
            fp[:, :, cpad:cpad + W] = f
            out = np.empty((32, XGROWS, NPLANE, WO), np.float32)
            for ci, cs in enumerate(range(-3, 4)):
                out[:, :, ci, :] = fp[:, :, cpad + cs:cpad + cs + 2 * WO:2]
            return out

        qpl = to_planes(qrow)   # [32, 262, 7, 128]
        dpl = to_planes(drow)
        # interleave channel pairs: [GPC, row, plane, c, wo]
        qi = np.empty((GPC, XGROWS, NPLANE, 2, WO), np.float32)
        di = np.empty((GPC, XGROWS, NPLANE, 2, WO), np.float32)
        for gl in range(GPC):
            qi[gl, :, :, 0] = qpl[2 * gl]
            qi[gl, :, :, 1] = qpl[2 * gl + 1]
            di[gl, :, :, 0] = dpl[2 * gl]
            di[gl, :, :, 1] = dpl[2 * gl + 1]
        qfv = np.ascontiguousarray(
            qi.reshape(GPC, XGROWS, RPITCH)).astype(np.float16)
        dfv = np.ascontiguousarray(
            di.reshape(GPC, XGROWS, RPITCH)).astype(np.float16)

        # wg: [GPC, oc, k, c, wo]
        wg = w_deform.reshape(G, 2, 2, KK)[gs]      # [GPC, oc, c, k]
        wgv = np.ascontiguousarray(
            np.broadcast_to(
                wg.transpose(0, 1, 3, 2)[:, :, :, :, None],
                (GPC, 2, KK, 2, WO)).reshape(GPC, 2 * KK * CW)
        ).astype(np.float16)
        in_maps.append({
            "xconv": xconv,
            "qf": qfv,
            "df": dfv,
            "wconv": wconv,
            "biasv": ball,
            "wgv": wgv,
        })
    return in_maps


def kernel(**inputs):
    from concourse.bass_utils import run_bass_kernel_spmd

    if "prog" not in _PROGRAM_CACHE:
        _PROGRAM_CACHE["prog"] = build_program()
    nc = _PROGRAM_CACHE["prog"]
    in_maps = _host_prep(inputs)
    res = run_bass_kernel_spmd(nc, in_maps, list(range(NCORES)),
                               trace=bool(int(os.environ.get("BEV_TRACE", "0"))))
    _PROGRAM_CACHE["last_result"] = res
    out = np.empty((B, C, HO, WO), dtype=np.float32)
    for core in range(NCORES):
        b = core // 4
        q = core % 4
        out[b, q * 32:(q + 1) * 32] = res.results[core]["y"]
    return out
